# revision 26
# baseline (speedup 1.0000x reference)
"""KeypointFlowLoss Trainium2 kernel.

The loss only reads each flow at the K keypoint pixels that the reference
scatters into the ground-truth flow image (every other pixel has gt == 0 and
mask == 0), so instead of streaming 5 x [16,2,512,512] f32 from HBM we gather
exactly the needed pixels with indirect DMA and reduce on-chip.

Sharding: data-parallel over the batch dim — core c owns batches
[2c, 2c+2). The five flows are stacked into one DRAM tensor per core so a
single indirect DMA gathers all 5 flows x 2 channels per keypoint (indirect
DMAs are gpsimd-only, so separate tensors would serialize five gathers on the
Pool queue). Each core emits [34, 6] per-keypoint partials ([5 masked EPEs,
mask]); the host all-reduces the partials and applies the weighted division.

Critical path: one input DMA (SP; keypoints + constant offset-bias table
in a single [34,14] tile) -> 2 fused int ops on DVE for gather offsets ->
one indirect gather (Pool) -> 4 DVE ops for the masked squared EPE ->
ACT sqrt -> out DMA (ACT). disp/mask are computed off-path during the
DMA windows. vs the naive scatter+stream approach this reads ~1KB instead
of ~42MB per core.
"""

import numpy as np

import concourse.bacc as bacc
import concourse.bass as bass
import concourse.mybir as mybir
import concourse.tile as tile
from concourse.bass import IndirectOffsetOnAxis
from concourse.bass_utils import run_bass_kernel_spmd

B, CH, H, W = 16, 2, 512, 512
K = 17
NF = 5
NCORES = 8
BL = B // NCORES          # batches per core
NP = BL * K               # keypoints per core
GAMMA = 0.8
LOSS_WEIGHT = 1.0

HW = H * W
CHW = CH * HW
FLAT = NF * BL * CHW      # elements in the per-core stacked flow tensor

F32 = mybir.dt.float32
I32 = mybir.dt.int32

_PROGRAM = None
_RUN_KWARGS = {}      # test harness can set {"trace": True} to profile
_LAST_RESULTS = None

# constant element-offset bias: cadd[p, f, c] = f*BL*CHW + (p >= K)*CHW + c*HW
_CADD = (
    np.arange(NF, dtype=np.int64)[None, :, None] * (BL * CHW)
    + (np.arange(NP, dtype=np.int64)[:, None, None] >= K) * CHW
    + np.arange(CH, dtype=np.int64)[None, None, :] * HW
).astype(np.int32).reshape(NP, NF * CH)


def _build_program():
    nc = bacc.Bacc(None, target_bir_lowering=False)

    flows = nc.dram_tensor("flows", [NF, BL, CH, H, W], F32, kind="ExternalInput")
    # host pre-arranges kps as [NP, 14] rows of [x0, y0, x1, y1, cadd*10]
    # where cadd is the constant offset-bias table (shape-derived, not data)
    kps = nc.dram_tensor("kps", [NP, 4 + NF * CH], I32, kind="ExternalInput")
    out = nc.dram_tensor("out", [NP, NF + 1], F32, kind="ExternalOutput")

    with tile.TileContext(nc) as tc:
        with tc.tile_pool(name="sbuf", bufs=1) as sb:
            # single input DMA: keypoints + constant table in one tile.
            # SP queue: the ACT queue's Sqrt table load would delay it.
            kt = sb.tile([NP, 4 + NF * CH], I32)
            nc.sync.dma_start(out=kt[:], in_=kps[:])
            cadd = kt[:, 4:4 + NF * CH].rearrange("p (f c) -> p f c", c=CH)

            # ---- critical path: element offsets y*W + (x + cadd) ----
            xc = sb.tile([NP, NF, CH], I32)
            nc.vector.tensor_tensor(
                out=xc[:],
                in0=kt[:, 0:1].unsqueeze(2).broadcast_to([NP, NF, CH]),
                in1=cadd, op=mybir.AluOpType.add)
            offs = sb.tile([NP, NF, CH], I32)
            nc.vector.scalar_tensor_tensor(
                out=offs[:],
                in0=kt[:, 1:2].unsqueeze(2).broadcast_to([NP, NF, CH]),
                scalar=W, in1=xc[:],
                op0=mybir.AluOpType.mult,
                op1=mybir.AluOpType.add)

            # ---- single gather: all 5 flows x 2 channels per keypoint ----
            g = sb.tile([NP, NF, CH], F32)
            flat = bass.AP(flows, 0, [[1, FLAT], [1, 1]])
            nc.gpsimd.indirect_dma_start(
                out=g[:], out_offset=None, in_=flat,
                in_offset=IndirectOffsetOnAxis(ap=offs[:], axis=0))

            # ---- off-path during the gather: disp and mask ----
            kf = sb.tile([NP, 4], F32)
            nc.vector.tensor_copy(out=kf[:], in_=kt[:, 0:4])  # int -> float, exact
            disp = sb.tile([NP, 2], F32)
            nc.vector.tensor_tensor(out=disp[:], in0=kf[:, 2:4], in1=kf[:, 0:2],
                                    op=mybir.AluOpType.subtract)
            dsq0 = sb.tile([NP, 2], F32)
            nc.vector.tensor_tensor(out=dsq0[:], in0=disp[:], in1=disp[:],
                                    op=mybir.AluOpType.mult)
            r2 = sb.tile([NP, 1], F32)
            nc.vector.tensor_tensor(out=r2[:], in0=dsq0[:, 0:1], in1=dsq0[:, 1:2],
                                    op=mybir.AluOpType.add)
            # vcols = [5 masked EPE columns, mask]; mask written first (off-path)
            vcols = sb.tile([NP, NF + 1], F32)
            nc.vector.tensor_scalar(out=vcols[:, NF:NF + 1], in0=r2[:],
                                    scalar1=0.0, scalar2=None,
                                    op0=mybir.AluOpType.is_gt)

            # ---- post-gather: squared EPE and pair-sum ----
            d = sb.tile([NP, NF, CH], F32)
            nc.vector.tensor_tensor(
                out=d[:], in0=g[:],
                in1=disp[:].unsqueeze(1).broadcast_to([NP, NF, CH]),
                op=mybir.AluOpType.subtract)
            nc.vector.tensor_tensor(out=d[:], in0=d[:], in1=d[:],
                                    op=mybir.AluOpType.mult)
            # pair-sum dx^2 + dy^2 via stride-2 column slices
            s = sb.tile([NP, NF], F32)
            nc.vector.tensor_tensor(out=s[:], in0=d[:, :, 0], in1=d[:, :, 1],
                                    op=mybir.AluOpType.add)
            # sqrt(s * mask) == sqrt(s) * mask for mask in {0, 1}
            nc.vector.tensor_tensor(
                out=s[:], in0=s[:],
                in1=vcols[:, NF:NF + 1].broadcast_to([NP, NF]),
                op=mybir.AluOpType.mult)
            nc.scalar.activation(out=vcols[:, 0:NF], in_=s[:],
                                 func=mybir.ActivationFunctionType.Sqrt)

            nc.scalar.dma_start(out=out[:], in_=vcols[:])

    nc.finalize()
    return nc


def _get_program():
    global _PROGRAM
    if _PROGRAM is None:
        _PROGRAM = _build_program()
    return _PROGRAM


def kernel(**inputs):
    flows = [np.asarray(inputs[f"flow{i}"], dtype=np.float32) for i in range(NF)]
    kps = np.ascontiguousarray(np.asarray(inputs["kps"], dtype=np.int32))

    nc = _get_program()

    in_maps = []
    for c in range(NCORES):
        sl = slice(c * BL, (c + 1) * BL)
        # [BL,2,K,2] -> [BL,K,2,2] -> [NP,4] rows of [x0,y0,x1,y1],
        # then append the constant offset-bias table -> [NP,14]
        kps_r = np.concatenate(
            [kps[sl].transpose(0, 2, 1, 3).reshape(NP, 4), _CADD], axis=1)
        in_maps.append({
            "flows": np.stack([flows[i][sl] for i in range(NF)]),
            "kps": np.ascontiguousarray(kps_r),
        })

    results = run_bass_kernel_spmd(nc, in_maps, core_ids=list(range(NCORES)),
                                   **_RUN_KWARGS)
    globals()["_LAST_RESULTS"] = results

    # all-reduce the per-keypoint partials: [NCORES*NP, 6]
    total = np.zeros(NF + 1, dtype=np.float32)
    for r in results.results:
        total += r["out"].reshape(NP, NF + 1).astype(np.float32).sum(axis=0)

    sums, cnt = total[:NF], total[NF]
    weights = (np.float32(GAMMA) ** np.arange(NF - 1, -1, -1, dtype=np.float32))
    means = sums / np.float32(cnt)
    loss = np.float32(np.sum(weights * means, dtype=np.float32) * np.float32(LOSS_WEIGHT))
    return np.asarray(loss, dtype=np.float32)


# revision 29
# speedup vs baseline: 1.0800x; 1.0800x over previous
"""KeypointFlowLoss Trainium2 kernel.

The loss only reads each flow at the K keypoint pixels that the reference
scatters into the ground-truth flow image (every other pixel has gt == 0 and
mask == 0), so instead of streaming 5 x [16,2,512,512] f32 from HBM we gather
exactly the needed pixels with indirect DMA and reduce on-chip.

Sharding: data-parallel over the batch dim — core c owns batches
[2c, 2c+2). The five flows are stacked into one DRAM tensor per core so a
single indirect DMA gathers all 5 flows x 2 channels per keypoint (indirect
DMAs are gpsimd-only, so separate tensors would serialize five gathers on the
Pool queue). Each core emits [34, 6] per-keypoint partials ([5 masked EPEs,
mask]); the host all-reduces the partials and applies the weighted division.

Critical path: one input DMA (SP; keypoints + constant offset-bias table
in a single [34,14] tile) -> 2 fused int ops on DVE for gather offsets ->
one indirect gather (Pool) -> 4 DVE ops for the masked squared EPE ->
ACT sqrt -> out DMA (ACT). disp/mask are computed off-path during the
DMA windows. vs the naive scatter+stream approach this reads ~1KB instead
of ~42MB per core.

The program is hand-scheduled raw Bass (manual semaphores, no TileContext):
at ~15 instructions the tile framework's entry/exit barriers and drains
cost ~0.9us, which this avoids. Intra-engine RAW hazards need explicit
semaphores too (deep pipelines, no interlocks) — s_v counts DVE ops.
"""

import numpy as np

import concourse.bacc as bacc
import concourse.bass as bass
import concourse.mybir as mybir
from concourse.bass import IndirectOffsetOnAxis
from concourse.bass_utils import run_bass_kernel_spmd

B, CH, H, W = 16, 2, 512, 512
K = 17
NF = 5
NCORES = 8
BL = B // NCORES          # batches per core
NP = BL * K               # keypoints per core
GAMMA = 0.8
LOSS_WEIGHT = 1.0

HW = H * W
CHW = CH * HW
FLAT = NF * BL * CHW      # elements in the per-core stacked flow tensor

F32 = mybir.dt.float32
I32 = mybir.dt.int32

_PROGRAM = None
_RUN_KWARGS = {}      # test harness can set {"trace": True} to profile
_LAST_RESULTS = None

# constant element-offset bias: cadd[p, f, c] = f*BL*CHW + (p >= K)*CHW + c*HW
_CADD = (
    np.arange(NF, dtype=np.int64)[None, :, None] * (BL * CHW)
    + (np.arange(NP, dtype=np.int64)[:, None, None] >= K) * CHW
    + np.arange(CH, dtype=np.int64)[None, None, :] * HW
).astype(np.int32).reshape(NP, NF * CH)


def _build_program():
    nc = bacc.Bacc(None, target_bir_lowering=False)

    flows = nc.dram_tensor("flows", [NF, BL, CH, H, W], F32, kind="ExternalInput")
    # host pre-arranges kps as [NP, 14] rows of [x0, y0, x1, y1, cadd*10]
    # where cadd is the constant offset-bias table (shape-derived, not data)
    kps = nc.dram_tensor("kps", [NP, 4 + NF * CH], I32, kind="ExternalInput")
    out = nc.dram_tensor("out", [NP, NF + 1], F32, kind="ExternalOutput")

    AL = mybir.AluOpType
    ctx = []

    def sbuf(name, shape, dt):
        cm = nc.sbuf_tensor(name, shape, dt)
        ctx.append(cm)
        return cm.__enter__()

    def sem(name):
        cm = nc.semaphore(name)
        ctx.append(cm)
        return cm.__enter__()

    kt = sbuf("kt", [NP, 4 + NF * CH], I32)
    xc = sbuf("xc", [NP, NF * CH], I32)
    offs = sbuf("offs", [NP, NF * CH], I32)
    g = sbuf("g", [NP, NF * CH], F32)
    kf = sbuf("kf", [NP, 4], F32)
    disp = sbuf("disp", [NP, 2], F32)
    dsq0 = sbuf("dsq0", [NP, 2], F32)
    r2 = sbuf("r2", [NP, 1], F32)
    vcols = sbuf("vcols", [NP, NF + 1], F32)
    s = sbuf("s", [NP, NF], F32)

    s_k = sem("s_k")   # input DMA done
    s_o = sem("s_o")   # offsets ready
    s_g = sem("s_g")   # gather done
    s_m = sem("s_m")   # mask column written
    s_s = sem("s_s")   # masked squared EPE ready
    s_q = sem("s_q")   # sqrt done
    s_d = sem("s_d")   # out DMA done
    s_v = sem("s_v")   # DVE intra-engine op counter

    def ap(t, off, pat):
        return bass.AP(t, off, pat)

    # input DMA on SP (ACT would stall it behind the Sqrt table load)
    nc.sync.dma_start(out=ap(kt, 0, [[14, NP], [1, 14]]),
                      in_=kps[:]).then_inc(s_k, 16)

    # ---- DVE: offsets y*W + (x + cadd), then off-path disp/mask ----
    cadd3 = ap(kt, 4, [[14, NP], [2, NF], [1, CH]])
    x_b = ap(kt, 0, [[14, NP], [0, NF], [0, CH]])
    y_b = ap(kt, 1, [[14, NP], [0, NF], [0, CH]])
    xc3 = ap(xc, 0, [[10, NP], [2, NF], [1, CH]])
    offs3 = ap(offs, 0, [[10, NP], [2, NF], [1, CH]])
    g3 = ap(g, 0, [[10, NP], [2, NF], [1, CH]])

    nc.vector.wait_ge(s_k, 16)
    nc.vector.tensor_tensor(out=xc3, in0=x_b, in1=cadd3,
                            op=AL.add).then_inc(s_v, 1)
    nc.vector.wait_ge(s_v, 1)
    nc.vector.scalar_tensor_tensor(out=offs3, in0=y_b, scalar=W, in1=xc3,
                                   op0=AL.mult, op1=AL.add).then_inc(s_o, 1)
    nc.vector.tensor_copy(out=ap(kf, 0, [[4, NP], [1, 4]]),
                          in_=ap(kt, 0, [[14, NP], [1, 4]])).then_inc(s_v, 1)
    nc.vector.wait_ge(s_v, 2)
    nc.vector.tensor_tensor(out=ap(disp, 0, [[2, NP], [1, 2]]),
                            in0=ap(kf, 2, [[4, NP], [1, 2]]),
                            in1=ap(kf, 0, [[4, NP], [1, 2]]),
                            op=AL.subtract).then_inc(s_v, 1)
    nc.vector.wait_ge(s_v, 3)
    nc.vector.tensor_tensor(out=ap(dsq0, 0, [[2, NP], [1, 2]]),
                            in0=ap(disp, 0, [[2, NP], [1, 2]]),
                            in1=ap(disp, 0, [[2, NP], [1, 2]]),
                            op=AL.mult).then_inc(s_v, 1)
    nc.vector.wait_ge(s_v, 4)
    nc.vector.tensor_tensor(out=ap(r2, 0, [[1, NP], [1, 1]]),
                            in0=ap(dsq0, 0, [[2, NP], [1, 1]]),
                            in1=ap(dsq0, 1, [[2, NP], [1, 1]]),
                            op=AL.add).then_inc(s_v, 1)
    nc.vector.wait_ge(s_v, 5)
    nc.vector.tensor_scalar(out=ap(vcols, NF, [[NF + 1, NP], [1, 1]]),
                            in0=ap(r2, 0, [[1, NP], [1, 1]]), scalar1=0.0,
                            scalar2=None, op0=AL.is_gt).then_inc(s_m, 1)

    # ---- Pool: single indirect gather of all 5 flows x 2 channels ----
    nc.gpsimd.wait_ge(s_o, 1)
    flat = bass.AP(flows, 0, [[1, FLAT], [1, 1]])
    nc.gpsimd.indirect_dma_start(
        out=g3, out_offset=None, in_=flat,
        in_offset=IndirectOffsetOnAxis(ap=offs3, axis=0)).then_inc(s_g, 16)

    # ---- DVE: masked squared EPE (in place in g) ----
    disp_b = ap(disp, 0, [[2, NP], [0, NF], [1, CH]])
    nc.vector.wait_ge(s_g, 16)
    nc.vector.tensor_tensor(out=g3, in0=g3, in1=disp_b,
                            op=AL.subtract).then_inc(s_v, 1)
    nc.vector.wait_ge(s_v, 6)
    nc.vector.tensor_tensor(out=g3, in0=g3, in1=g3,
                            op=AL.mult).then_inc(s_v, 1)
    sA = ap(s, 0, [[NF, NP], [1, NF]])
    nc.vector.wait_ge(s_v, 7)
    nc.vector.tensor_tensor(out=sA, in0=ap(g, 0, [[10, NP], [2, NF]]),
                            in1=ap(g, 1, [[10, NP], [2, NF]]),
                            op=AL.add).then_inc(s_v, 1)
    # sqrt(s * mask) == sqrt(s) * mask for mask in {0, 1}
    mask_b = ap(vcols, NF, [[NF + 1, NP], [0, NF]])
    nc.vector.wait_ge(s_v, 8)
    nc.vector.tensor_tensor(out=sA, in0=sA, in1=mask_b,
                            op=AL.mult).then_inc(s_s, 1)

    # ---- ACT: sqrt, then the out DMA from the same queue ----
    nc.scalar.wait_ge(s_s, 1)
    nc.scalar.activation(out=ap(vcols, 0, [[NF + 1, NP], [1, NF]]), in_=sA,
                         func=mybir.ActivationFunctionType.Sqrt).then_inc(s_q, 1)
    nc.scalar.wait_ge(s_q, 1)
    nc.scalar.wait_ge(s_m, 1)
    nc.scalar.dma_start(out=out[:],
                        in_=ap(vcols, 0, [[NF + 1, NP], [1, NF + 1]])
                        ).then_inc(s_d, 16)
    nc.scalar.wait_ge(s_d, 16)  # program ends when the result is in DRAM

    nc.finalize()
    for cm in reversed(ctx):
        cm.__exit__(None, None, None)
    return nc


def _get_program():
    global _PROGRAM
    if _PROGRAM is None:
        _PROGRAM = _build_program()
    return _PROGRAM


def kernel(**inputs):
    flows = [np.asarray(inputs[f"flow{i}"], dtype=np.float32) for i in range(NF)]
    kps = np.ascontiguousarray(np.asarray(inputs["kps"], dtype=np.int32))

    nc = _get_program()

    in_maps = []
    for c in range(NCORES):
        sl = slice(c * BL, (c + 1) * BL)
        # [BL,2,K,2] -> [BL,K,2,2] -> [NP,4] rows of [x0,y0,x1,y1],
        # then append the constant offset-bias table -> [NP,14]
        kps_r = np.concatenate(
            [kps[sl].transpose(0, 2, 1, 3).reshape(NP, 4), _CADD], axis=1)
        in_maps.append({
            "flows": np.stack([flows[i][sl] for i in range(NF)]),
            "kps": np.ascontiguousarray(kps_r),
        })

    results = run_bass_kernel_spmd(nc, in_maps, core_ids=list(range(NCORES)),
                                   **_RUN_KWARGS)
    globals()["_LAST_RESULTS"] = results

    # all-reduce the per-keypoint partials: [NCORES*NP, 6]
    total = np.zeros(NF + 1, dtype=np.float32)
    for r in results.results:
        total += r["out"].reshape(NP, NF + 1).astype(np.float32).sum(axis=0)

    sums, cnt = total[:NF], total[NF]
    weights = (np.float32(GAMMA) ** np.arange(NF - 1, -1, -1, dtype=np.float32))
    means = sums / np.float32(cnt)
    loss = np.float32(np.sum(weights * means, dtype=np.float32) * np.float32(LOSS_WEIGHT))
    return np.asarray(loss, dtype=np.float32)
